# revision 1
# baseline (speedup 1.0000x reference)
"""Trainium2 Bass kernel for nn_MultiHeadHCGAttention.

Math notes (exact restructuring of the reference):
  The key_padding_mask replaces the ENTIRE key feature row with -1e9 BEFORE
  the K projection (v is NOT masked). Hence every masked key position s in
  batch b has the SAME projected K row:
      Kmask[n] = -1e9 * sum_h Wk[n,h,:] + bk[n]   (data independent)
  All masked keys share one score z = Q.Kmask/sqrt(dk) with |z| ~ 1e9.
  In fp32 softmax the output per (query q, head n) is therefore either
    - mean of V over the masked key positions  if z > max unmasked score
      (uniform softmax over the identical-score masked keys)
    - standard softmax over unmasked keys      otherwise (masked weights
      underflow to exactly 0 in fp32)
  The boundary band has probability ~1e-7 per query -> decided by sign(z),
  computed exactly on the host in fp64 (z = q @ (Wq@Kmask) + bq.Kmask).

  Device computes bf16 attention over the gathered unmasked keys only
  (normal O(1) magnitudes); rows whose head chose the mask branch are zeroed
  on device (notchoose scale) and the contribution
  ubar[b,n] = (mean_masked V[b,n]) @ Wo_n is added on the host in fp64.

Sharding: 8 cores = (batch b in 0..3) x (query half). No collectives.
"""

import math
import sys

if "/opt/trn_rl_repo" not in sys.path:
    sys.path.insert(0, "/opt/trn_rl_repo")

import ml_dtypes
import numpy as np

import concourse.bacc as bacc
import concourse.tile as tile
from concourse import mybir
from concourse.bass_utils import run_bass_kernel_spmd

S, B, H = 2048, 4, 1024
NH, DK = 8, 128
NHDK = NH * DK
NEG = -1.0e9
NCORES = 8
HT = H // 128  # 8 H-tiles

bf16 = mybir.dt.bfloat16
f32 = mybir.dt.float32
npbf16 = ml_dtypes.bfloat16

_PROG_CACHE: dict = {}


def build_program(Sq: int, UP: int):
    """Emit the per-core SPMD program. Sq = queries per core, UP = padded
    unmasked-key count (multiple of 128)."""
    NKT = (UP + 127) // 128
    ktiles = [(o, min(128, UP - o)) for o in range(0, UP, 128)]
    QC = Sq // 512  # 512-wide query chunks
    # key free-dim chunks for the K projection
    kchunks = []
    o = 0
    while o < UP:
        w = min(512, UP - o)
        kchunks.append((o, w))
        o += w

    nc = bacc.Bacc("TRN2", target_bir_lowering=False, debug=False)

    d_qT = nc.dram_tensor("qT", [H, Sq], bf16, kind="ExternalInput")
    d_kuT = nc.dram_tensor("kuT", [H, UP], bf16, kind="ExternalInput")
    d_vuT = nc.dram_tensor("vuT", [H, UP], bf16, kind="ExternalInput")
    d_wq = nc.dram_tensor("wq", [H, NHDK], bf16, kind="ExternalInput")
    d_wk = nc.dram_tensor("wk", [H, NHDK], bf16, kind="ExternalInput")
    d_wv = nc.dram_tensor("wv", [H, NHDK], bf16, kind="ExternalInput")
    d_wo = nc.dram_tensor("wo", [NHDK, H], bf16, kind="ExternalInput")
    d_bq = nc.dram_tensor("bq", [DK, NH], f32, kind="ExternalInput")
    d_bk = nc.dram_tensor("bk", [DK, NH], f32, kind="ExternalInput")
    d_bv = nc.dram_tensor("bv", [1, NHDK], bf16, kind="ExternalInput")
    d_bo = nc.dram_tensor("bo", [128, HT], f32, kind="ExternalInput")
    d_padb = nc.dram_tensor("padb", [128, NKT], f32, kind="ExternalInput")
    d_nch = nc.dram_tensor("nch", [1, NH * Sq], f32, kind="ExternalInput")
    d_yT = nc.dram_tensor("yT", [H, Sq], f32, kind="ExternalOutput")

    SCALE = 1.0 / math.sqrt(DK)

    with tile.TileContext(nc) as tc:
        with (
            tc.tile_pool(name="const", bufs=1) as const,
            tc.tile_pool(name="kv", bufs=1) as kvp,
            tc.tile_pool(name="qp", bufs=1) as qp,
            tc.tile_pool(name="vg", bufs=2) as vgp,
            tc.tile_pool(name="exp", bufs=3) as expp,
            tc.tile_pool(name="sc", bufs=3) as scp,
            tc.tile_pool(name="bc", bufs=2) as bcp,
            tc.tile_pool(name="yt", bufs=3) as ytp,
            tc.tile_pool(name="ps_proj", bufs=3, space="PSUM") as ps_proj,
            tc.tile_pool(name="ps_pv", bufs=3, space="PSUM") as ps_pv,
            tc.tile_pool(name="ps_d", bufs=2, space="PSUM") as ps_d,
        ):
            # ---- constant loads (split per H-tile, compute-first order) ----
            qT = const.tile([128, HT, Sq], bf16)
            kuT = const.tile([128, HT, UP], bf16)
            vuT = const.tile([128, HT, UP], bf16)
            wq = const.tile([128, HT, NHDK], bf16)
            wk = const.tile([128, HT, NHDK], bf16)
            wv = const.tile([128, HT, NHDK], bf16)
            wo = const.tile([128, NH, H], bf16)
            r_qT = d_qT[:].rearrange("(t p) s -> p t s", p=128)
            r_kuT = d_kuT[:].rearrange("(t p) u -> p t u", p=128)
            r_vuT = d_vuT[:].rearrange("(t p) u -> p t u", p=128)
            r_wq = d_wq[:].rearrange("(t p) d -> p t d", p=128)
            r_wk = d_wk[:].rearrange("(t p) d -> p t d", p=128)
            r_wv = d_wv[:].rearrange("(t p) d -> p t d", p=128)
            r_wo = d_wo[:].rearrange("(n p) h -> p n h", p=128)
            bq = const.tile([128, NH], f32)
            bk = const.tile([128, NH], f32)
            bv = const.tile([1, NHDK], bf16)
            bo = const.tile([128, HT], f32)
            padb = const.tile([128, NKT], f32)

            ones_mat = const.tile([128, 128], bf16)
            nc.vector.memset(ones_mat[:], 1.0)
            out_all = const.tile([128, NH, Sq], bf16)  # per-head attn outputs
            # Single SP HWDGE ring, strict consumption order: the ring is
            # FIFO and splits each transfer across all 16 SDMA engines, so
            # one ring gets full bandwidth AND priority ordering.
            for ht in range(HT):  # V-projection group 0 inputs first
                nc.sync.dma_start(vuT[:, ht, :], r_vuT[:, ht, :])
                nc.sync.dma_start(wv[:, ht, 0:512], r_wv[:, ht, 0:512])
                if ht == 0:
                    nc.sync.dma_start(bq[:], d_bq[:])
                    nc.sync.dma_start(bk[:], d_bk[:])
                    nc.sync.dma_start(bv[:], d_bv[:])
                    nc.sync.dma_start(bo[:], d_bo[:])
                    nc.sync.dma_start(padb[:], d_padb[:])
            for ht in range(HT):  # then K projection inputs
                nc.sync.dma_start(kuT[:, ht, :], r_kuT[:, ht, :])
                nc.sync.dma_start(wk[:, ht, :], r_wk[:, ht, :])
            for ht in range(HT):  # then Q
                nc.sync.dma_start(qT[:, ht, :], r_qT[:, ht, :])
                nc.sync.dma_start(wq[:, ht, :], r_wq[:, ht, :])
            for ht in range(HT):  # V group 1 weights
                nc.sync.dma_start(wv[:, ht, 512:1024], r_wv[:, ht, 512:1024])
            for n in range(NH):
                nc.sync.dma_start(wo[:, n, :], r_wo[:, n, :])

            def vproj(g):
                bvb = bcp.tile([128, 512], bf16, tag="bvb")
                nc.gpsimd.dma_start(
                    bvb[:], d_bv[0:1, g * 512 : (g + 1) * 512].to_broadcast([128, 512])
                )
                vg = vgp.tile([128, NKT, 512], bf16)
                for kt, (ko, klen) in enumerate(ktiles):
                    pv = ps_proj.tile([128, 512], f32, tag="proj")
                    for ht in range(HT):
                        nc.tensor.matmul(
                            pv[:klen],
                            vuT[:, ht, ko : ko + klen],
                            wv[:, ht, g * 512 : (g + 1) * 512],
                            start=(ht == 0),
                            stop=(ht == HT - 1),
                        )
                    nc.vector.tensor_add(vg[:klen, kt, :], pv[:klen], bvb[:klen])
                return vg

            def kproj(n):
                ksb = kvp.tile([128, UP], bf16, tag=f"ksb{n}")
                for o, w in kchunks:
                    pk = ps_proj.tile([128, 512], f32, tag="proj")
                    for ht in range(HT):
                        nc.tensor.matmul(
                            pk[:, :w],
                            wk[:, ht, n * 128 : (n + 1) * 128],
                            kuT[:, ht, o : o + w],
                            start=(ht == 0),
                            stop=(ht == HT - 1),
                        )
                    nc.vector.tensor_scalar_add(
                        ksb[:, o : o + w], pk[:, :w], bk[:, n : n + 1]
                    )
                return ksb

            def qproj(n):
                qsb = qp.tile([128, Sq], bf16, tag=f"qsb{n}")
                for qc in range(QC):
                    pq = ps_proj.tile([128, 512], f32, tag="proj")
                    for ht in range(HT):
                        nc.tensor.matmul(
                            pq[:],
                            wq[:, ht, n * 128 : (n + 1) * 128],
                            qT[:, ht, qc * 512 : (qc + 1) * 512],
                            start=(ht == 0),
                            stop=(ht == HT - 1),
                        )
                    nc.vector.tensor_scalar_add(
                        qsb[:, qc * 512 : (qc + 1) * 512], pq[:], bq[:, n : n + 1]
                    )
                return qsb

            def attention(n, ksb, qsb, vg):
                for qc in range(QC):
                    qsl = slice(qc * 512, (qc + 1) * 512)
                    scbn = bcp.tile([128, 512], bf16, tag="scb")
                    nc.gpsimd.dma_start(
                        scbn[:],
                        d_nch[
                            0:1, n * Sq + qc * 512 : n * Sq + (qc + 1) * 512
                        ].to_broadcast([128, 512]),
                    )
                    ppv = ps_pv.tile([128, 512], f32)
                    pd = ps_d.tile([128, 512], f32)
                    for kt, (ko, klen) in enumerate(ktiles):
                        ps = ps_proj.tile([128, 512], f32, tag="proj")
                        nc.tensor.matmul(
                            ps[:klen],
                            ksb[:, ko : ko + klen],
                            qsb[:, qsl],
                            start=True,
                            stop=True,
                        )
                        e = expp.tile([128, 512], bf16)
                        nc.scalar.activation(
                            out=e[:klen],
                            in_=ps[:klen],
                            func=mybir.ActivationFunctionType.Exp,
                            bias=padb[:klen, kt : kt + 1],
                            scale=SCALE,
                        )
                        nc.tensor.matmul(
                            ppv[:],
                            vg[:klen, kt, n % 4 * 128 : (n % 4 + 1) * 128],
                            e[:klen],
                            start=(kt == 0),
                            stop=(kt == NKT - 1),
                        )
                        nc.tensor.matmul(
                            pd[:],
                            ones_mat[:klen],
                            e[:klen],
                            start=(kt == 0),
                            stop=(kt == NKT - 1),
                        )
                    rec = scp.tile([128, 512], f32, tag="rec")
                    nc.vector.reciprocal(rec[:], pd[:])
                    scl = scp.tile([128, 512], f32, tag="scl")
                    nc.vector.tensor_mul(scl[:], rec[:], scbn[:])
                    nc.vector.tensor_mul(out_all[:, n, qsl], ppv[:], scl[:])

            # phase 1: all projections (paced by the input DMA stream)
            vg0 = vproj(0)
            ks = {}
            qs = {}
            for n in range(4):
                ks[n] = kproj(n)
            for n in range(4):
                qs[n] = qproj(n)
            for n in range(4, NH):
                ks[n] = kproj(n)
            for n in range(4, NH):
                qs[n] = qproj(n)
            vg1 = vproj(1)
            # phase 2: attention (no DMA dependencies)
            for n in range(NH):
                attention(n, ks[n], qs[n], vg0 if n < 4 else vg1)

            # ---- output projection: yT[h, q] = sum_n Wo_n.T @ out_n + bo ----
            for qc in range(QC):
                for ht in range(HT):
                    py = ps_proj.tile([128, 512], f32, tag="proj")
                    for n in range(NH):
                        nc.tensor.matmul(
                            py[:],
                            wo[:, n, ht * 128 : (ht + 1) * 128],
                            out_all[:, n, qc * 512 : (qc + 1) * 512],
                            start=(n == 0),
                            stop=(n == NH - 1),
                        )
                    yt = ytp.tile([128, 512], f32)
                    nc.vector.tensor_scalar_add(yt[:], py[:], bo[:, ht : ht + 1])
                    nc.sync.dma_start(
                        d_yT[:].rearrange("(t p) s -> t p s", p=128)[
                            ht, :, qc * 512 : (qc + 1) * 512
                        ],
                        yt[:],
                    )

    nc.compile()
    return nc


def _prepare(query, key, value, key_padding_mask, Wq, bq, Wk, bk, Wv, bv, Wo, bo):
    """Host-side prep: mask constants (fp64), gather/transpose, per-core maps."""
    mask = np.asarray(key_padding_mask)
    q64 = np.asarray(query, np.float64)
    Wq64 = np.asarray(Wq, np.float64)
    Wk64 = np.asarray(Wk, np.float64)
    Wv64 = np.asarray(Wv, np.float64)
    Wo64 = np.asarray(Wo, np.float64)

    # shared projected row of all masked keys, per head
    kmask = NEG * Wk64.sum(axis=1) + np.asarray(bk, np.float64)  # [NH, DK]

    # z sign per (s, b, n):  z = q . (Wq[n] @ kmask[n]) + bq[n].kmask[n]
    wz = np.einsum("nhd,nd->hn", Wq64, kmask)  # [H, NH]
    cz = np.einsum("nd,nd->n", np.asarray(bq, np.float64), kmask)  # [NH]
    z = q64.reshape(S * B, H) @ wz + cz  # [S*B, NH]
    choose = (z > 0).reshape(S, B, NH)

    # mask-branch output: mean of (unmasked-data) V over masked key positions
    v64 = np.asarray(value, np.float64)  # [S, B, H]
    vbar_feat = np.stack(
        [
            v64[mask[b], b, :].mean(axis=0)
            if mask[b].any()
            else np.zeros(H)
            for b in range(B)
        ]
    )  # [B, H]
    for b in range(B):
        if not mask[b].any():
            choose[:, b, :] = False  # no masked keys -> no mask branch
        elif mask[b].all():
            # all keys masked: identical scores -> uniform softmax -> Vbar
            choose[:, b, :] = True
    vbar = (
        np.einsum("bh,nhd->bnd", vbar_feat, Wv64) + np.asarray(bv, np.float64)[None]
    )  # [B, NH, DK]
    ubar = np.einsum(
        "bnd,ndh->bnh", vbar, Wo64.reshape(NH, DK, H)
    )  # [B, NH, H]

    # correction added on host for mask-branch rows
    ycorr = np.einsum("sbn,bnh->sbh", choose.astype(np.float64), ubar)

    # gather unmasked keys per batch
    idx = [np.nonzero(~mask[b])[0] for b in range(B)]
    umax = max(max(len(i) for i in idx), 1)
    UP = umax
    NKT = (UP + 127) // 128

    Wq_d = np.ascontiguousarray(
        np.asarray(Wq).transpose(1, 0, 2).reshape(H, NHDK)
    ).astype(npbf16)
    Wk_d = np.ascontiguousarray(
        np.asarray(Wk).transpose(1, 0, 2).reshape(H, NHDK)
    ).astype(npbf16)
    Wv_d = np.ascontiguousarray(
        np.asarray(Wv).transpose(1, 0, 2).reshape(H, NHDK)
    ).astype(npbf16)
    Wo_d = np.asarray(Wo, np.float32).astype(npbf16)
    bq_d = np.ascontiguousarray(np.asarray(bq, np.float32).T)  # [DK, NH]
    bk_d = np.ascontiguousarray(np.asarray(bk, np.float32).T)
    bv_d = np.asarray(bv, np.float32).reshape(1, NHDK).astype(npbf16)
    bo_d = np.ascontiguousarray(
        np.asarray(bo, np.float32).reshape(HT, 128).T
    )  # [128, HT]

    Sq = S // 2
    in_maps = []
    for core in range(NCORES):
        b, half = divmod(core, 2)
        qo = half * Sq
        ii = idx[b]
        u = len(ii)
        kuT = np.zeros((H, UP), npbf16)
        kuT[:, :u] = np.asarray(key[ii, b, :], np.float32).T.astype(npbf16)
        vuT = np.zeros((H, UP), npbf16)
        vuT[:, :u] = np.asarray(value[ii, b, :], np.float32).T.astype(npbf16)
        qT = np.ascontiguousarray(
            np.asarray(query[qo : qo + Sq, b, :], np.float32).T
        ).astype(npbf16)
        padb = np.zeros((128, NKT), np.float32)
        flat = np.arange(NKT * 128).reshape(NKT, 128).T  # [128, NKT] key index
        padb[flat >= max(u, 1)] = -30000.0  # keep >=1 live key (denom > 0)
        nch = np.ascontiguousarray(
            (~choose[qo : qo + Sq, b, :]).T.astype(np.float32)
        ).reshape(1, NH * Sq)
        in_maps.append(
            {
                "qT": qT,
                "kuT": kuT,
                "vuT": vuT,
                "wq": Wq_d,
                "wk": Wk_d,
                "wv": Wv_d,
                "wo": Wo_d,
                "bq": bq_d,
                "bk": bk_d,
                "bv": bv_d,
                "bo": bo_d,
                "padb": padb,
                "nch": nch,
            }
        )
    return in_maps, ycorr, Sq, UP


def run(inputs: dict, trace: bool = False):
    in_maps, ycorr, Sq, UP = _prepare(**inputs)
    key_ = (Sq, UP)
    if key_ not in _PROG_CACHE:
        _PROG_CACHE[key_] = build_program(Sq, UP)
    nc = _PROG_CACHE[key_]
    res = run_bass_kernel_spmd(nc, in_maps, list(range(NCORES)), trace=trace)
    y = np.empty((S, B, H), np.float32)
    for core in range(NCORES):
        b, half = divmod(core, 2)
        qo = half * Sq
        y[qo : qo + Sq, b, :] = res.results[core]["yT"].T
    y += ycorr.astype(np.float32)
    return y, res


def kernel(**inputs) -> np.ndarray:
    y, _ = run(inputs, trace=False)
    return y



# revision 7
# speedup vs baseline: 1.1934x; 1.1934x over previous
"""Trainium2 Bass kernel for nn_MultiHeadHCGAttention.

Math notes (exact restructuring of the reference):
  The key_padding_mask replaces the ENTIRE key feature row with -1e9 BEFORE
  the K projection (v is NOT masked). Hence every masked key position s in
  batch b has the SAME projected K row:
      Kmask[n] = -1e9 * sum_h Wk[n,h,:] + bk[n]   (data independent)
  All masked keys share one score z = Q.Kmask/sqrt(dk) with |z| ~ 1e9.
  In fp32 softmax the output per (query q, head n) is therefore either
    - mean of V over the masked key positions  if z > max unmasked score
      (uniform softmax over the identical-score masked keys)
    - standard softmax over unmasked keys      otherwise (masked weights
      underflow to exactly 0 in fp32)
  The boundary band has probability ~1e-7 per query -> decided by sign(z),
  computed exactly on the host in fp64 (z = q @ (Wq@Kmask) + bq.Kmask).

  Device computes bf16 attention over the gathered unmasked keys only
  (normal O(1) magnitudes); rows whose head chose the mask branch are zeroed
  on device (notchoose scale) and the contribution
  ubar[b,n] = (mean_masked V[b,n]) @ Wo_n is added on the host in fp64.

Sharding: 8 cores = (batch b in 0..3) x (query half). No collectives.
"""

import math
import sys

if "/opt/trn_rl_repo" not in sys.path:
    sys.path.insert(0, "/opt/trn_rl_repo")

import ml_dtypes
import numpy as np

import concourse.bacc as bacc
import concourse.tile as tile
from concourse import mybir
from concourse.bass_utils import run_bass_kernel_spmd

S, B, H = 2048, 4, 1024
NH, DK = 8, 128
NHDK = NH * DK
NEG = -1.0e9
NCORES = 8
HT = H // 128  # 8 H-tiles

bf16 = mybir.dt.bfloat16
f32 = mybir.dt.float32
npbf16 = ml_dtypes.bfloat16

_PROG_CACHE: dict = {}


def build_program(Sq: int, UP: int):
    """Emit the per-core SPMD program. Sq = queries per core, UP = padded
    unmasked-key count (multiple of 128)."""
    NKT = (UP + 127) // 128
    ktiles = [(o, min(128, UP - o)) for o in range(0, UP, 128)]
    QC = Sq // 512  # 512-wide query chunks
    # key free-dim chunks for the K projection
    kchunks = []
    o = 0
    while o < UP:
        w = min(512, UP - o)
        kchunks.append((o, w))
        o += w

    nc = bacc.Bacc("TRN2", target_bir_lowering=False, debug=False)

    d_qT = nc.dram_tensor("qT", [H, Sq], bf16, kind="ExternalInput")
    d_kuT = nc.dram_tensor("kuT", [H, UP], bf16, kind="ExternalInput")
    d_vuT = nc.dram_tensor("vuT", [H, UP], bf16, kind="ExternalInput")
    d_wq = nc.dram_tensor("wq", [H, NHDK], bf16, kind="ExternalInput")
    d_wk = nc.dram_tensor("wk", [H, NHDK], bf16, kind="ExternalInput")
    d_wv = nc.dram_tensor("wv", [H, NHDK], bf16, kind="ExternalInput")
    d_wo = nc.dram_tensor("wo", [NHDK, H], bf16, kind="ExternalInput")
    d_bq = nc.dram_tensor("bq", [DK, NH], f32, kind="ExternalInput")
    d_bk = nc.dram_tensor("bk", [DK, NH], f32, kind="ExternalInput")
    d_bv = nc.dram_tensor("bv", [1, NHDK], bf16, kind="ExternalInput")
    d_bo = nc.dram_tensor("bo", [128, HT], f32, kind="ExternalInput")
    d_padb = nc.dram_tensor("padb", [128, NKT], f32, kind="ExternalInput")
    d_nch = nc.dram_tensor("nch", [1, NH * Sq], f32, kind="ExternalInput")
    d_yT = nc.dram_tensor("yT", [H, Sq], bf16, kind="ExternalOutput")

    SCALE = 1.0 / math.sqrt(DK)

    with tile.TileContext(nc) as tc:
        with (
            tc.tile_pool(name="const", bufs=1) as const,
            tc.tile_pool(name="kv", bufs=1) as kvp,
            tc.tile_pool(name="qp", bufs=1) as qp,
            tc.tile_pool(name="vg", bufs=2) as vgp,
            tc.tile_pool(name="exp", bufs=3) as expp,
            tc.tile_pool(name="es", bufs=2) as esp,
            tc.tile_pool(name="sc", bufs=3) as scp,
            tc.tile_pool(name="bc", bufs=2) as bcp,
            tc.tile_pool(name="yt", bufs=3) as ytp,
            tc.tile_pool(name="ps_proj", bufs=3, space="PSUM") as ps_proj,
            tc.tile_pool(name="ps_pv", bufs=3, space="PSUM") as ps_pv,
            tc.tile_pool(name="ps_d", bufs=2, space="PSUM") as ps_d,
        ):
            # ---- constant loads (split per H-tile, compute-first order) ----
            qT = const.tile([128, HT, Sq], bf16)
            kuT = const.tile([128, HT, UP], bf16)
            vuT = const.tile([128, HT, UP], bf16)
            wq = const.tile([128, HT, NHDK], bf16)
            wk = const.tile([128, HT, NHDK], bf16)
            wv = const.tile([128, HT, NHDK], bf16)
            wo = const.tile([128, NH, H], bf16)
            r_qT = d_qT[:].rearrange("(t p) s -> p t s", p=128)
            r_kuT = d_kuT[:].rearrange("(t p) u -> p t u", p=128)
            r_vuT = d_vuT[:].rearrange("(t p) u -> p t u", p=128)
            r_wq = d_wq[:].rearrange("(t p) d -> p t d", p=128)
            r_wk = d_wk[:].rearrange("(t p) d -> p t d", p=128)
            r_wv = d_wv[:].rearrange("(t p) d -> p t d", p=128)
            r_wo = d_wo[:].rearrange("(n p) h -> p n h", p=128)
            bq = const.tile([128, NH], f32)
            bk = const.tile([128, NH], f32)
            bv = const.tile([1, NHDK], bf16)
            bo = const.tile([128, HT], f32)
            padb = const.tile([128, NKT], f32)

            ones_mat = const.tile([128, 128], bf16)
            nc.vector.memset(ones_mat[:], 1.0)
            out_all = const.tile([128, NH, Sq], bf16)  # per-head attn outputs
            # Single SP HWDGE ring, strict consumption order: the ring is
            # FIFO and splits each transfer across all 16 SDMA engines, so
            # one ring gets full bandwidth AND priority ordering.
            for ht in range(HT):  # V-projection group 0 inputs first
                nc.sync.dma_start(vuT[:, ht, :], r_vuT[:, ht, :])
                nc.sync.dma_start(wv[:, ht, 0:512], r_wv[:, ht, 0:512])
                if ht == 0:
                    nc.sync.dma_start(bq[:], d_bq[:])
                    nc.sync.dma_start(bk[:], d_bk[:])
                    nc.sync.dma_start(bv[:], d_bv[:])
                    nc.sync.dma_start(bo[:], d_bo[:])
                    nc.sync.dma_start(padb[:], d_padb[:])
            for ht in range(HT):  # then K projection inputs
                nc.sync.dma_start(kuT[:, ht, :], r_kuT[:, ht, :])
                nc.sync.dma_start(wk[:, ht, :], r_wk[:, ht, :])
            for ht in range(HT):  # then Q
                nc.sync.dma_start(qT[:, ht, :], r_qT[:, ht, :])
                nc.sync.dma_start(wq[:, ht, :], r_wq[:, ht, :])
            for ht in range(HT):  # V group 1 weights
                nc.sync.dma_start(wv[:, ht, 512:1024], r_wv[:, ht, 512:1024])
            for n in range(NH):
                nc.sync.dma_start(wo[:, n, :], r_wo[:, n, :])

            def vproj(g):
                bvb = bcp.tile([128, 512], bf16, tag="bvb")
                nc.gpsimd.dma_start(
                    bvb[:], d_bv[0:1, g * 512 : (g + 1) * 512].to_broadcast([128, 512])
                )
                vg = vgp.tile([128, NKT, 512], bf16)
                for kt, (ko, klen) in enumerate(ktiles):
                    pv = ps_proj.tile([128, 512], f32, tag="proj")
                    for ht in range(HT):
                        nc.tensor.matmul(
                            pv[:klen],
                            vuT[:, ht, ko : ko + klen],
                            wv[:, ht, g * 512 : (g + 1) * 512],
                            start=(ht == 0),
                            stop=(ht == HT - 1),
                        )
                    nc.vector.tensor_add(vg[:klen, kt, :], pv[:klen], bvb[:klen])
                return vg

            def kproj(n):
                ksb = kvp.tile([128, UP], bf16, tag=f"ksb{n}")
                for o, w in kchunks:
                    pk = ps_proj.tile([128, 512], f32, tag="proj")
                    for ht in range(HT):
                        nc.tensor.matmul(
                            pk[:, :w],
                            wk[:, ht, n * 128 : (n + 1) * 128],
                            kuT[:, ht, o : o + w],
                            start=(ht == 0),
                            stop=(ht == HT - 1),
                        )
                    nc.vector.tensor_scalar_add(
                        ksb[:, o : o + w], pk[:, :w], bk[:, n : n + 1]
                    )
                return ksb

            def qproj(n):
                qsb = qp.tile([128, Sq], bf16, tag=f"qsb{n}")
                for qc in range(QC):
                    pq = ps_proj.tile([128, 512], f32, tag="proj")
                    for ht in range(HT):
                        nc.tensor.matmul(
                            pq[:],
                            wq[:, ht, n * 128 : (n + 1) * 128],
                            qT[:, ht, qc * 512 : (qc + 1) * 512],
                            start=(ht == 0),
                            stop=(ht == HT - 1),
                        )
                    nc.vector.tensor_scalar_add(
                        qsb[:, qc * 512 : (qc + 1) * 512], pq[:], bq[:, n : n + 1]
                    )
                return qsb

            def attention(n, ksb, qsb, vg):
                for qc in range(QC):
                    qsl = slice(qc * 512, (qc + 1) * 512)
                    scbn = bcp.tile([128, 512], bf16, tag="scb")
                    nc.gpsimd.dma_start(
                        scbn[:],
                        d_nch[
                            0:1, n * Sq + qc * 512 : n * Sq + (qc + 1) * 512
                        ].to_broadcast([128, 512]),
                    )
                    ppv = ps_pv.tile([128, 512], f32)
                    esum = esp.tile([128, 512], bf16)
                    e0 = None
                    for kt, (ko, klen) in enumerate(ktiles):
                        ps = ps_proj.tile([128, 512], f32, tag="proj")
                        nc.tensor.matmul(
                            ps[:klen],
                            ksb[:, ko : ko + klen],
                            qsb[:, qsl],
                            start=True,
                            stop=True,
                        )
                        e = expp.tile([128, 512], bf16)
                        nc.scalar.activation(
                            out=e[:klen],
                            in_=ps[:klen],
                            func=mybir.ActivationFunctionType.Exp,
                            bias=padb[:klen, kt : kt + 1],
                            scale=SCALE,
                        )
                        nc.tensor.matmul(
                            ppv[:],
                            vg[:klen, kt, n % 4 * 128 : (n % 4 + 1) * 128],
                            e[:klen],
                            start=(kt == 0),
                            stop=(kt == NKT - 1),
                        )
                        # running tile-sum of e on DVE (bf16); partition-sum
                        # happens in the single ones-matmul below.
                        if kt == 0:
                            e0 = e
                        elif kt == 1:
                            nc.vector.tensor_add(esum[:klen], e0[:klen], e[:klen])
                        else:
                            nc.vector.tensor_add(esum[:klen], esum[:klen], e[:klen])
                    pd = ps_d.tile([128, 512], f32)
                    dsrc = esum if NKT > 1 else e0
                    nc.tensor.matmul(
                        pd[:], ones_mat[:], dsrc[:], start=True, stop=True
                    )
                    rec = scp.tile([128, 512], f32, tag="rec")
                    nc.vector.reciprocal_approx_fast(rec[:], pd[:])
                    scl = scp.tile([128, 512], f32, tag="scl")
                    nc.vector.tensor_mul(scl[:], rec[:], scbn[:])
                    nc.vector.tensor_mul(out_all[:, n, qsl], ppv[:], scl[:])

            # phase 1: all projections (paced by the input DMA stream)
            vg0 = vproj(0)
            ks = {}
            qs = {}
            for n in range(4):
                ks[n] = kproj(n)
            for n in range(4):
                qs[n] = qproj(n)
            for n in range(4, NH):
                ks[n] = kproj(n)
            for n in range(4, NH):
                qs[n] = qproj(n)
            vg1 = vproj(1)
            # phase 2: attention (no DMA dependencies)
            for n in range(NH):
                attention(n, ks[n], qs[n], vg0 if n < 4 else vg1)

            # ---- output projection: yT[h, q] = sum_n Wo_n.T @ out_n + bo ----
            for qc in range(QC):
                for ht in range(HT):
                    py = ps_proj.tile([128, 512], f32, tag="proj")
                    for n in range(NH):
                        nc.tensor.matmul(
                            py[:],
                            wo[:, n, ht * 128 : (ht + 1) * 128],
                            out_all[:, n, qc * 512 : (qc + 1) * 512],
                            start=(n == 0),
                            stop=(n == NH - 1),
                        )
                    yt = ytp.tile([128, 512], bf16)
                    nc.vector.tensor_scalar_add(yt[:], py[:], bo[:, ht : ht + 1])
                    nc.sync.dma_start(
                        d_yT[:].rearrange("(t p) s -> t p s", p=128)[
                            ht, :, qc * 512 : (qc + 1) * 512
                        ],
                        yt[:],
                    )

    nc.compile()
    return nc


def _prepare(query, key, value, key_padding_mask, Wq, bq, Wk, bk, Wv, bv, Wo, bo):
    """Host-side prep: mask constants (fp64), gather/transpose, per-core maps."""
    mask = np.asarray(key_padding_mask)
    q64 = np.asarray(query, np.float64)
    Wq64 = np.asarray(Wq, np.float64)
    Wk64 = np.asarray(Wk, np.float64)
    Wv64 = np.asarray(Wv, np.float64)
    Wo64 = np.asarray(Wo, np.float64)

    # shared projected row of all masked keys, per head
    kmask = NEG * Wk64.sum(axis=1) + np.asarray(bk, np.float64)  # [NH, DK]

    # z sign per (s, b, n):  z = q . (Wq[n] @ kmask[n]) + bq[n].kmask[n]
    wz = np.einsum("nhd,nd->hn", Wq64, kmask)  # [H, NH]
    cz = np.einsum("nd,nd->n", np.asarray(bq, np.float64), kmask)  # [NH]
    z = q64.reshape(S * B, H) @ wz + cz  # [S*B, NH]
    choose = (z > 0).reshape(S, B, NH)

    # mask-branch output: mean of (unmasked-data) V over masked key positions
    v64 = np.asarray(value, np.float64)  # [S, B, H]
    vbar_feat = np.stack(
        [
            v64[mask[b], b, :].mean(axis=0)
            if mask[b].any()
            else np.zeros(H)
            for b in range(B)
        ]
    )  # [B, H]
    for b in range(B):
        if not mask[b].any():
            choose[:, b, :] = False  # no masked keys -> no mask branch
        elif mask[b].all():
            # all keys masked: identical scores -> uniform softmax -> Vbar
            choose[:, b, :] = True
    vbar = (
        np.einsum("bh,nhd->bnd", vbar_feat, Wv64) + np.asarray(bv, np.float64)[None]
    )  # [B, NH, DK]
    ubar = np.einsum(
        "bnd,ndh->bnh", vbar, Wo64.reshape(NH, DK, H)
    )  # [B, NH, H]

    # correction added on host for mask-branch rows
    ycorr = np.einsum("sbn,bnh->sbh", choose.astype(np.float64), ubar)

    # gather unmasked keys per batch
    idx = [np.nonzero(~mask[b])[0] for b in range(B)]
    umax = max(max(len(i) for i in idx), 1)
    UP = umax
    NKT = (UP + 127) // 128

    Wq_d = np.ascontiguousarray(
        np.asarray(Wq).transpose(1, 0, 2).reshape(H, NHDK)
    ).astype(npbf16)
    Wk_d = np.ascontiguousarray(
        np.asarray(Wk).transpose(1, 0, 2).reshape(H, NHDK)
    ).astype(npbf16)
    Wv_d = np.ascontiguousarray(
        np.asarray(Wv).transpose(1, 0, 2).reshape(H, NHDK)
    ).astype(npbf16)
    Wo_d = np.asarray(Wo, np.float32).astype(npbf16)
    bq_d = np.ascontiguousarray(np.asarray(bq, np.float32).T)  # [DK, NH]
    bk_d = np.ascontiguousarray(np.asarray(bk, np.float32).T)
    bv_d = np.asarray(bv, np.float32).reshape(1, NHDK).astype(npbf16)
    bo_d = np.ascontiguousarray(
        np.asarray(bo, np.float32).reshape(HT, 128).T
    )  # [128, HT]

    Sq = S // 2
    in_maps = []
    for core in range(NCORES):
        b, half = divmod(core, 2)
        qo = half * Sq
        ii = idx[b]
        u = len(ii)
        kuT = np.zeros((H, UP), npbf16)
        kuT[:, :u] = np.asarray(key[ii, b, :], np.float32).T.astype(npbf16)
        vuT = np.zeros((H, UP), npbf16)
        vuT[:, :u] = np.asarray(value[ii, b, :], np.float32).T.astype(npbf16)
        qT = np.ascontiguousarray(
            np.asarray(query[qo : qo + Sq, b, :], np.float32).T
        ).astype(npbf16)
        padb = np.zeros((128, NKT), np.float32)
        flat = np.arange(NKT * 128).reshape(NKT, 128).T  # [128, NKT] key index
        padb[flat >= max(u, 1)] = -30000.0  # keep >=1 live key (denom > 0)
        nch = np.ascontiguousarray(
            (~choose[qo : qo + Sq, b, :]).T.astype(np.float32)
        ).reshape(1, NH * Sq)
        in_maps.append(
            {
                "qT": qT,
                "kuT": kuT,
                "vuT": vuT,
                "wq": Wq_d,
                "wk": Wk_d,
                "wv": Wv_d,
                "wo": Wo_d,
                "bq": bq_d,
                "bk": bk_d,
                "bv": bv_d,
                "bo": bo_d,
                "padb": padb,
                "nch": nch,
            }
        )
    return in_maps, ycorr, Sq, UP


def run(inputs: dict, trace: bool = False):
    in_maps, ycorr, Sq, UP = _prepare(**inputs)
    key_ = (Sq, UP)
    if key_ not in _PROG_CACHE:
        _PROG_CACHE[key_] = build_program(Sq, UP)
    nc = _PROG_CACHE[key_]
    res = run_bass_kernel_spmd(nc, in_maps, list(range(NCORES)), trace=trace)
    y = np.empty((S, B, H), np.float32)
    for core in range(NCORES):
        b, half = divmod(core, 2)
        qo = half * Sq
        y[qo : qo + Sq, b, :] = res.results[core]["yT"].astype(np.float32).T
    y += ycorr.astype(np.float32)
    return y, res


def kernel(**inputs) -> np.ndarray:
    y, _ = run(inputs, trace=False)
    return y

